# revision 9
# baseline (speedup 1.0000x reference)
"""GAT (3-layer, DGL GATConv-style) on 8 Trainium2 NeuronCores.

Self-contained kernel: kernel(**inputs) takes the full unsharded inputs
(features [50000,256] f32, src/dst [800000] i32, per-layer W/al/ar/b),
distributes across 8 cores (dst-slab graph partition), runs one Bass
kernel launch per GAT layer, and returns the full [50000, 64] output.

Device-side design (per core, per layer):
  phase A: each core computes the node table for its OWN slab only:
           row = [feat int8 x256 (per-row scale) | el f16 x4 |
                  scale f16 | er f16 x4] (274B in a 512B-stride row),
           in natural node order; feat is quantized to int8 on the
           scalar engine with a per-row reciprocal scale from the DVE
           absmax.  One 8-core DRAM AllGather then replicates the
           full 50176-row table to every core.
  phase B: per-edge rows gathered with dma_gather (266B payload =
           int8 feat + f16 el + f16 scale, int16 node-id indices into
           lo/hi table halves, 4 SWDGE queues);
           t = el[src]+er[dst]  (er via one-hot PE matmul, el via DVE);
           ex = max(exp(t), exp(0.2t))  == exp(leaky_relu(t, 0.2));
           weighted scatter-aggregation as PE matmul:
              psum[64dst, 260] += onehot_ed(fp8).T @ [feat*(ex*s) | ex]
           with the int8 dequant scale s folded into the edge weights;
           epilogue: rst = acc/den + (h+b); final layer computes
           mean_h relu(rst) via scalar-engine relu with scale=1/4.
Graph structure (tile schedule, one-hot matrices, gather indices) is
precomputed on the host once and reused for all three layers.
"""

import sys

sys.path.insert(0, "/opt/trn_rl_repo")

import inspect
import textwrap

import numpy as np
import ml_dtypes

import concourse.bacc as bacc
import concourse.bass as bass
import concourse.mybir as mybir
import concourse.tile as tile

F32 = mybir.dt.float32
F16 = mybir.dt.float16
BF16 = mybir.dt.bfloat16
F8 = mybir.dt.float8e4
I16 = mybir.dt.int16
I8 = mybir.dt.int8
U8 = mybir.dt.uint8

BF = ml_dtypes.bfloat16
E4M3 = ml_dtypes.float8_e4m3

# --- patch dma_gather: drop the (transpose-only) elem_size%256 assert ---
_src = textwrap.dedent(inspect.getsource(bass.BassGpSimd.dma_gather))
_src = _src.replace("elem_size_bytes > 0 and elem_size_bytes % 256 == 0",
                    "elem_size_bytes > 0")
_src = _src.replace("def dma_gather(", "def _dma_gather_relaxed(", 1)
_ns = dict(bass.__dict__)
exec(compile(_src, "patched_dma_gather", "exec"), _ns)
bass.BassGpSimd.dma_gather_relaxed = _ns["_dma_gather_relaxed"]


class Cfg:
    def __init__(self, N, E, D, H, DH, n_cores, win=64, kblk=16, grp=6,
                 out_heads_mean=False):
        self.N = N
        self.E = E
        self.D = D
        self.H = H
        self.DH = DH
        self.C = n_cores
        self.WIN = win      # dst nodes per window (psum group)
        self.KBLK = kblk    # edge-tiles per compute block
        self.GRP = grp      # windows per gather group
        slab = -(-N // n_cores)
        slab = -(-slab // win) * win
        while (slab * n_cores) % 128:
            slab += win
        self.NSLAB = slab
        self.NPAD = slab * n_cores
        self.NW = slab // win
        assert self.NPAD % 128 == 0
        assert self.NSLAB % 128 == 0
        self.NT = self.NPAD // 128
        self.TS = self.NSLAB // 128          # own-slab tiles per core
        self.HALF = min(32768, self.NPAD)    # lo table rows (node id < HALF)
        self.HIBASE = self.NPAD - self.HALF  # hi rows indexed from HIBASE
        self.WXC = D + 2 * H                 # Wx columns: feat | el | er
        self.ROWB = D + 2 * H + 2            # gathered payload bytes
        self.RSTB = 512                      # table row stride bytes
        self.out_heads_mean = out_heads_mean


def plan_edges(cfg, src, dst):
    """Common tile schedule + per-core edge tensors.

    Tiles are grouped: per window, lo(A)-half tiles then hi(B)-half
    tiles (half = which slab-half the src node's table row lives in);
    windows are grouped into gather groups of GRP windows.  Edges
    within a (core, window, half) segment are sorted by src row for
    gather locality.
    """
    C, WIN, NW, NSLAB, GRP = cfg.C, cfg.WIN, cfg.NW, cfg.NSLAB, cfg.GRP
    core_of = dst // NSLAB
    dloc = dst % NSLAB
    win_of = dloc // WIN

    deg = np.zeros(cfg.NPAD, dtype=np.int64)
    np.add.at(deg, dst, 1)
    zdeg = deg == 0

    half_of = (src >= cfg.HALF).astype(np.int64)  # 0 = lo, 1 = hi
    row_of = np.where(half_of == 0, src, src - cfg.HIBASE)

    # counts per (core, window, half)
    cnt = np.zeros((C, NW, 2), dtype=np.int64)
    np.add.at(cnt, (core_of, win_of, half_of), 1)
    # fake edges (src row 0 -> A) for zero-degree dsts
    zz = np.nonzero(zdeg)[0]
    np.add.at(cnt, (zz // NSLAB, (zz % NSLAB) // WIN, np.zeros(len(zz), np.int64)), 1)

    t_lo = -(-cnt[:, :, 0].max(axis=0) // 128)
    t_hi = -(-cnt[:, :, 1].max(axis=0) // 128)
    # every window needs >= 1 tile total (fakes guarantee lo>=1 when needed)
    t_lo = np.maximum(t_lo, (t_lo + t_hi == 0).astype(np.int64))

    # global slot ids: grouped by (group, half, window)
    wslots = [[] for _ in range(NW)]
    hslots = {}          # (w, half) -> list of slot ids
    groups = []
    T = 0
    for g in range(-(-NW // GRP)):
        ws = list(range(g * GRP, min((g + 1) * GRP, NW)))
        slots = []
        lo0 = T
        for w in ws:
            hslots[(w, 0)] = list(range(T, T + int(t_lo[w])))
            wslots[w] += hslots[(w, 0)]
            slots += [(w, 0)] * int(t_lo[w])
            T += int(t_lo[w])
        lo1 = T
        for w in ws:
            hslots[(w, 1)] = list(range(T, T + int(t_hi[w])))
            wslots[w] += hslots[(w, 1)]
            slots += [(w, 1)] * int(t_hi[w])
            T += int(t_hi[w])
        hi1 = T
        groups.append(dict(slots=slots, lo=(lo0, lo1), hi=(lo1, hi1)))

    eidx = np.zeros((C, 128, T * 8), dtype=np.int16)
    ohe = np.zeros((C, 128, T * WIN), dtype=E4M3)
    ohd = np.zeros((C, 64, T * 128), dtype=np.float16)

    key = (core_of * NW + win_of) * 2 + half_of
    order = np.lexsort((row_of, key))        # sort by src row within segments
    r_sorted = row_of[order]
    d_sorted = dst[order]
    cw = key[order]
    starts = np.searchsorted(cw, np.arange(C * NW * 2))
    ends = np.searchsorted(cw, np.arange(C * NW * 2) + 1)

    # wrap map: index i of a tile -> (row i%16, col i//16)
    wrap_r = np.arange(128) % 16
    wrap_c = np.arange(128) // 16

    for c in range(C):
        for w in range(NW):
            base_d = c * NSLAB + w * WIN
            for half in (0, 1):
                kk = (c * NW + w) * 2 + half
                i0, i1 = starts[kk], ends[kk]
                rr_ = list(r_sorted[i0:i1])
                dd = list((d_sorted[i0:i1] - base_d))
                if half == 0:
                    for dl in range(WIN):
                        if zdeg[base_d + dl]:
                            rr_.append(0)
                            dd.append(dl)
                sl_ids = hslots[(w, half)]
                nslots = len(sl_ids) * 128
                assert len(rr_) <= nslots, (c, w, half, len(rr_), nslots)
                npad = nslots - len(rr_)
                rr_ += [0] * npad
                dd += [-1] * npad
                rows = np.asarray(rr_, dtype=np.int64)
                dd = np.asarray(dd, dtype=np.int64)
                for j, t in enumerate(sl_ids):
                    rr = rows[j * 128:(j + 1) * 128]
                    ddj = dd[j * 128:(j + 1) * 128]
                    eidx[c, wrap_r, t * 8 + wrap_c] = rr.astype(np.int16)
                    p = np.nonzero(ddj >= 0)[0]
                    ohe[c, p, t * WIN + ddj[p]] = E4M3(1.0)
                    ohd[c, ddj[p], t * 128 + p] = np.float16(1.0)
    # replicate idx rows 0:16 across all 8 Q7 core groups
    for c in range(C):
        eidx[c] = np.tile(eidx[c, :16], (8, 1))
    return dict(groups=groups, wslots=wslots, T=T, eidx=eidx, ohe=ohe, ohd=ohd)


def pack_hT(cfg, h):
    """[NPAD, D] f32 -> [128, NT*D] f16 with cols (tile, kchunk, row):
    out[p, i*D + k*128 + q] = h[i*128 + q, k*128 + p]."""
    NT, D = cfg.NT, cfg.D
    KC = D // 128
    h4 = h.reshape(NT, 128, KC, 128).astype(np.float16)
    return np.ascontiguousarray(h4.transpose(3, 0, 2, 1).reshape(128, NT * D))


def make_wx(cfg, W, al, ar):
    H, DH = cfg.H, cfg.DH
    alm = np.zeros((cfg.D, H), dtype=np.float64)
    arm = np.zeros((cfg.D, H), dtype=np.float64)
    for h in range(H):
        alm[h * DH:(h + 1) * DH, h] = al[h]
        arm[h * DH:(h + 1) * DH, h] = ar[h]
    Wx = np.concatenate(
        [W.astype(np.float64), W.astype(np.float64) @ alm,
         W.astype(np.float64) @ arm], axis=1)
    return Wx.astype(np.float16)


def build_kernel(cfg, plan, final):
    D, H = cfg.D, cfg.H
    WIN, KBLK = cfg.WIN, cfg.KBLK
    ROWB, RSTB, WXC = cfg.ROWB, cfg.RSTB, cfg.WXC
    T = plan["T"]
    KC = D // 128
    DEN = D + H                            # 260 scatter columns
    # table row byte offsets: int8 feat | el f16 x4 | scale f16 | er f16 x4
    ELOFF = D                              # 256
    SCOFF = D + 2 * H                      # 264
    EROFF = D + 2 * H + 2                  # 266
    ERABS = 9                              # f16 slots el..er per row
    OUTD = cfg.DH if (cfg.out_heads_mean and final) else D

    nc = bacc.Bacc("TRN2", target_bir_lowering=False, debug=False,
                   enable_asserts=False, num_devices=cfg.C, num_swdge_queues=4)

    hTs = nc.dram_tensor("hTs", [128, cfg.TS * D], F16, kind="ExternalInput")
    Wx = nc.dram_tensor("Wx", [D, WXC], F16, kind="ExternalInput")
    hb = nc.dram_tensor("hb", [cfg.NSLAB, D], F16, kind="ExternalInput")
    eidx = nc.dram_tensor("eidx", [128, T * 8], I16, kind="ExternalInput")
    ohe_d = nc.dram_tensor("ohe", [128, T * WIN], F8, kind="ExternalInput")
    ohd_d = nc.dram_tensor("ohd", [64, T * 128], F16, kind="ExternalInput")
    out = nc.dram_tensor("out", [cfg.NSLAB, OUTD], F16, kind="ExternalOutput")
    tabS = nc.dram_tensor("tabS", [cfg.NSLAB, RSTB], U8, kind="Internal")
    tab = nc.dram_tensor("tab", [cfg.NPAD, RSTB], U8, kind="Internal",
                         addr_space="Shared")

    with tile.TileContext(nc) as tc:
        with (
            tc.tile_pool(name="const", bufs=1) as cpool,
            tc.tile_pool(name="grow", bufs=3) as gpool,
            tc.tile_pool(name="oh", bufs=6) as opool,
            tc.tile_pool(name="exg", bufs=3) as xpool,
            tc.tile_pool(name="tt", bufs=4) as tpool,
            tc.tile_pool(name="epi", bufs=4) as epool,
        ):
            wx0 = cpool.tile([128, WXC], F16, tag="wx0")
            wx1 = cpool.tile([128, WXC], F16, tag="wx1")
            nc.sync.dma_start(out=wx0[:], in_=Wx[0:128, :])
            nc.sync.dma_start(out=wx1[:], in_=Wx[128:256, :])
            eidx_t = cpool.tile([128, T * 8], I16, tag="eidx")
            nc.sync.dma_start(out=eidx_t[:], in_=eidx[:, :])

            # --- phase A: own slab, A-half then B-half ---
            erwin = cpool.tile([64, cfg.NW * H], F16, tag="erwin")
            with (
                tc.tile_pool(name="hblk", bufs=3) as hpool,
                tc.tile_pool(name="fblk", bufs=3) as fpool,
                tc.tile_pool(name="psA", bufs=2, space="PSUM") as psA,
            ):
                def phase_a(tabS, t0, nt):
                    AB = 8
                    tabS_r = tabS[:, :].rearrange("(i p) c -> p i c", p=128)
                    for blk0 in range(0, nt, AB):
                        blk1 = min(blk0 + AB, nt)
                        nb = blk1 - blk0
                        hblk = hpool.tile([128, AB * D], F16)
                        i0 = (t0 + blk0) * D
                        nc.sync.dma_start(
                            out=hblk[:, 0:nb * D], in_=hTs[:, i0:i0 + nb * D])
                        qblk = fpool.tile([128, AB * D], I8, tag="qblk")
                        eblk = fpool.tile([128, AB * ERABS], F16, tag="eblk")
                        for j in range(nb):
                            ps = psA.tile([128, WXC], F32)
                            for k in range(KC):
                                nc.tensor.matmul(
                                    out=ps[:],
                                    lhsT=hblk[:, j * D + k * 128:j * D + (k + 1) * 128],
                                    rhs=(wx0 if k == 0 else wx1)[:],
                                    start=(k == 0), stop=(k == KC - 1))
                            mx = fpool.tile([128, 1], F32, tag="mx")
                            nc.vector.tensor_reduce(
                                out=mx[:], in_=ps[:, 0:D],
                                axis=mybir.AxisListType.XYZW,
                                op=mybir.AluOpType.max,
                                apply_absolute_value=True)
                            ms = fpool.tile([128, 1], F32, tag="ms")
                            nc.vector.tensor_scalar_mul(
                                out=ms[:], in0=mx[:], scalar1=1.0 / 127.0)
                            rs = fpool.tile([128, 1], F32, tag="rs")
                            nc.vector.reciprocal(out=rs[:], in_=ms[:])
                            nc.scalar.activation(
                                out=qblk[:, j * D:(j + 1) * D],
                                in_=ps[:, 0:D],
                                func=mybir.ActivationFunctionType.Copy,
                                scale=rs[:])
                            nc.vector.tensor_copy(
                                out=eblk[:, j * ERABS + 4:j * ERABS + 5],
                                in_=ms[:])
                            nc.vector.tensor_copy(
                                out=eblk[:, j * ERABS:j * ERABS + 4],
                                in_=ps[:, D:D + H])
                            nc.vector.tensor_copy(
                                out=eblk[:, j * ERABS + 5:(j + 1) * ERABS],
                                in_=ps[:, D + H:D + 2 * H])
                        nc.sync.dma_start(
                            out=tabS_r[:, blk0:blk1, 0:D].bitcast(I8),
                            in_=qblk[:, 0:nb * D].rearrange("p (j c) -> p j c", c=D))
                        nc.sync.dma_start(
                            out=tabS_r[:, blk0:blk1, ELOFF:ELOFF + 2 * ERABS]
                            .bitcast(F16),
                            in_=eblk[:, 0:nb * ERABS].rearrange(
                                "p (j c) -> p j c", c=ERABS))

                phase_a(tabS, 0, cfg.TS)
                er_ap = (tabS[:, :].rearrange("(w d) c -> d w c", d=WIN)
                         [:, :, EROFF:EROFF + 2 * H].bitcast(F16))
                nc.sync.dma_start(
                    out=erwin[:].rearrange("p (w h) -> p w h", h=H),
                    in_=er_ap)
                nc.gpsimd.collective_compute(
                    kind="AllGather", op=mybir.AluOpType.bypass,
                    replica_groups=[list(range(cfg.C))],
                    ins=[tabS[:, :]], outs=[tab[:, :]])

            # --- phase B ---
            import contextlib
            _stk = contextlib.ExitStack()
            psT = _stk.enter_context(tc.tile_pool(name="psT", bufs=2, space="PSUM"))
            psB = _stk.enter_context(
                tc.tile_pool(name="psB", bufs=cfg.GRP, space="PSUM"))
            qn = [0]
            slot_to_win = {}
            for w in range(cfg.NW):
                for s in plan["wslots"][w]:
                    slot_to_win[s] = w

            hb_r = hb[:, :].rearrange("(w d) c -> d w c", d=WIN)
            out_r = out[:, :].rearrange("(w d) c -> d w c", d=WIN)
            for g, grp in enumerate(plan["groups"]):
                s_begin = grp["lo"][0]
                s_end = grp["hi"][1]
                nslot = s_end - s_begin
                w_lo = g * cfg.GRP
                w_hi = min((g + 1) * cfg.GRP, cfg.NW)
                nwg = w_hi - w_lo
                hbg = epool.tile([WIN, cfg.GRP * D], F16, tag="hbg")
                nc.sync.dma_start(
                    out=hbg[:, 0:nwg * D].rearrange("d (w c) -> d w c", c=D),
                    in_=hb_r[:, w_lo:w_hi, :])
                og = epool.tile([WIN, cfg.GRP * OUTD], F16, tag="og")
                grow = gpool.tile([128, nslot * ROWB], U8, tag="grow")
                CHUNK = 15  # tiles per gather call; small calls stay at
                            # pure desc-gen rate (no ring-reclaim stall)
                for half, (hh0, hh1) in (("lo", grp["lo"]), ("hi", grp["hi"])):
                    src_ap = (tab[0:cfg.HALF, 0:ROWB] if half == "lo"
                              else tab[cfg.HIBASE:cfg.NPAD, 0:ROWB])
                    for h0 in range(hh0, hh1, CHUNK):
                        h1 = min(h0 + CHUNK, hh1)
                        ni = (h1 - h0) * 128
                        nc.gpsimd.dma_gather_relaxed(
                            out_ap=grow[:, (h0 - s_begin) * ROWB:(h1 - s_begin) * ROWB]
                            .rearrange("p (t e) -> p t e", e=ROWB),
                            in_ap=src_ap,
                            idxs_ap=eidx_t[:, h0 * 8:h1 * 8],
                            num_idxs=ni, num_idxs_reg=ni,
                            elem_size=ROWB, elem_step=RSTB,
                            single_packet=False, queue_num=qn[0] % 4)
                        qn[0] += 1

                accs = {}
                open_w = {}
                for b0 in range(s_begin, s_end, KBLK):
                    b1 = min(b0 + KBLK, s_end)
                    k = b1 - b0
                    ohe_b = opool.tile([128, KBLK * WIN], F8, tag="ohe")
                    nc.scalar.dma_start(
                        out=ohe_b[:, 0:k * WIN],
                        in_=ohe_d[:, b0 * WIN:b1 * WIN])
                    ohd_b = opool.tile([64, KBLK * 128], F16, tag="ohd")
                    nc.scalar.dma_start(
                        out=ohd_b[:, 0:k * 128],
                        in_=ohd_d[:, b0 * 128:b1 * 128])
                    pst = psT.tile([128, KBLK * H], F32)
                    for j in range(k):
                        s = b0 + j
                        w = slot_to_win[s]
                        nc.tensor.matmul(
                            out=pst[:, j * H:(j + 1) * H],
                            lhsT=ohd_b[:, j * 128:(j + 1) * 128],
                            rhs=erwin[:, w * H:(w + 1) * H],
                            start=True, stop=True, skip_group_check=True)
                    grow_k = (grow[:, (b0 - s_begin) * ROWB:(b1 - s_begin) * ROWB]
                              .rearrange("p (t e) -> p t e", e=ROWB))
                    tsrc = tpool.tile([128, KBLK * H], BF16, tag="tt")
                    nc.vector.tensor_add(
                        out=tsrc[:, 0:k * H].rearrange("p (k h) -> p k h", h=H),
                        in0=pst[:, 0:k * H].rearrange("p (k h) -> p k h", h=H),
                        in1=grow_k[:, :, ELOFF:ELOFF + 2 * H].bitcast(F16))
                    xa = tpool.tile([128, KBLK * H], BF16, tag="xa")
                    xb = tpool.tile([128, KBLK * H], BF16, tag="xb")
                    nc.scalar.activation(
                        out=xa[:, 0:k * H], in_=tsrc[:, 0:k * H],
                        func=mybir.ActivationFunctionType.Exp)
                    nc.scalar.activation(
                        out=xb[:, 0:k * H], in_=tsrc[:, 0:k * H],
                        func=mybir.ActivationFunctionType.Exp, scale=0.2)
                    exg = xpool.tile([128, KBLK * DEN], BF16, tag="exg")
                    exg_k = exg[:, 0:k * DEN].rearrange("p (k c) -> p k c", c=DEN)
                    nc.vector.tensor_max(
                        out=exg_k[:, :, D:DEN],
                        in0=xa[:, 0:k * H].rearrange("p (k h) -> p k h", h=H),
                        in1=xb[:, 0:k * H].rearrange("p (k h) -> p k h", h=H))
                    # exs = ex * per-row scale  (dequant folded into weights)
                    exs = tpool.tile([128, KBLK * H], BF16, tag="exs")
                    nc.vector.tensor_mul(
                        out=exs[:, 0:k * H].rearrange("p (k h) -> p k h", h=H),
                        in0=exg_k[:, :, D:DEN],
                        in1=grow_k[:, :, SCOFF:SCOFF + 2].bitcast(F16)
                        .to_broadcast([128, k, H]))
                    feat_in = grow_k[:, :, 0:D].bitcast(I8).rearrange(
                        "p k (h f) -> p k h f", f=cfg.DH)
                    ex_in = (exs[:, 0:k * H].rearrange("p (k h) -> p k h", h=H)
                             .to_broadcast([128, k, H, cfg.DH]))
                    exg_out = exg_k[:, :, 0:D].rearrange(
                        "p k (h f) -> p k h f", f=cfg.DH)
                    nc.vector.tensor_mul(out=exg_out, in0=feat_in, in1=ex_in)

                    # scatter matmuls for this block
                    for j in range(k):
                        s = b0 + j
                        w = slot_to_win[s]
                        if w not in accs:
                            acc_w = psB.tile([WIN, DEN], F32, tag="acc")
                            accs[w] = acc_w
                            open_w[w] = 0
                        first = open_w[w] == 0
                        last = s == plan["wslots"][w][-1]
                        open_w[w] += 1
                        nc.tensor.matmul(
                            out=accs[w][:],
                            lhsT=ohe_b[:, j * WIN:(j + 1) * WIN],
                            rhs=exg[:, j * DEN:(j + 1) * DEN],
                            start=first, stop=last, skip_group_check=True)
                        if last:
                            acc = accs.pop(w)
                            wl = w - w_lo
                            rec = epool.tile([WIN, H], F32, tag="rec")
                            nc.vector.reciprocal(out=rec[:], in_=acc[:, D:DEN])
                            rst = epool.tile([WIN, D], F32, tag="rst")
                            for hh in range(H):
                                nc.scalar.activation(
                                    out=rst[:, hh * cfg.DH:(hh + 1) * cfg.DH],
                                    in_=acc[:, hh * cfg.DH:(hh + 1) * cfg.DH],
                                    func=mybir.ActivationFunctionType.Copy,
                                    scale=rec[:, hh:hh + 1])
                            if cfg.out_heads_mean and final:
                                nc.vector.tensor_add(
                                    out=rst[:], in0=rst[:],
                                    in1=hbg[:, wl * D:(wl + 1) * D])
                                rq = epool.tile([WIN, D], BF16, tag="rq")
                                nc.scalar.activation(
                                    out=rq[:], in_=rst[:],
                                    func=mybir.ActivationFunctionType.Relu,
                                    scale=0.25)
                                o1 = epool.tile([WIN, cfg.DH], F32, tag="o1")
                                nc.vector.tensor_add(
                                    out=o1[:], in0=rq[:, 0:cfg.DH],
                                    in1=rq[:, cfg.DH:2 * cfg.DH])
                                o2 = epool.tile([WIN, cfg.DH], F32, tag="o2")
                                nc.vector.tensor_add(
                                    out=o2[:], in0=rq[:, 2 * cfg.DH:3 * cfg.DH],
                                    in1=rq[:, 3 * cfg.DH:4 * cfg.DH])
                                nc.vector.tensor_add(
                                    out=og[:, wl * OUTD:(wl + 1) * OUTD],
                                    in0=o1[:], in1=o2[:])
                            else:
                                nc.vector.tensor_add(
                                    out=og[:, wl * OUTD:(wl + 1) * OUTD],
                                    in0=rst[:],
                                    in1=hbg[:, wl * D:(wl + 1) * D])
                nc.sync.dma_start(
                    out=out_r[:, w_lo:w_hi, :],
                    in_=og[:, 0:nwg * OUTD].rearrange("d (w c) -> d w c", c=OUTD))
            _stk.close()

    nc.compile()
    return nc


# ---------------------------------------------------------------------------
# kernel() entry point
# ---------------------------------------------------------------------------
_CACHE = {}

_N, _E, _D, _H, _DH = 50000, 800000, 256, 4, 64


def _get_built(src, dst):
    key = "built"
    if key in _CACHE:
        return _CACHE[key]
    cfg_mid = Cfg(_N, _E, _D, _H, _DH, n_cores=8, out_heads_mean=False)
    cfg_fin = Cfg(_N, _E, _D, _H, _DH, n_cores=8, out_heads_mean=True)
    plan = plan_edges(cfg_mid, src.astype(np.int64), dst.astype(np.int64))
    nc_mid = build_kernel(cfg_mid, plan, final=False)
    nc_fin = build_kernel(cfg_fin, plan, final=True)
    _CACHE[key] = (cfg_mid, cfg_fin, plan, nc_mid, nc_fin)
    return _CACHE[key]


def _make_in_maps(cfg, plan, h, W, al, ar, b):
    hTp = pack_hT(cfg, h)
    Wx = make_wx(cfg, W, al, ar)
    maps = []
    for c in range(cfg.C):
        sl = slice(c * cfg.NSLAB, (c + 1) * cfg.NSLAB)
        hb = (h[sl] + b[None, :]).astype(np.float16)
        hTs = hTp[:, c * cfg.TS * cfg.D:(c + 1) * cfg.TS * cfg.D]
        maps.append(dict(hTs=hTs, Wx=Wx, hb=hb, eidx=plan["eidx"][c],
                         ohe=plan["ohe"][c], ohd=plan["ohd"][c]))
    return maps


def _assemble(cfg, results, outd):
    out = np.zeros((cfg.NPAD, outd), dtype=np.float32)
    for c in range(cfg.C):
        out[c * cfg.NSLAB:(c + 1) * cfg.NSLAB] = results[c]["out"].astype(np.float32)
    out[cfg.N:] = 0.0
    return out


def kernel(features, src, dst, W0, al0, ar0, b0, W1, al1, ar1, b1,
           W2, al2, ar2, b2, _collect_exec_ns=None):
    from concourse.bass_utils import run_bass_kernel_spmd

    features = np.asarray(features, dtype=np.float32)
    src = np.asarray(src)
    dst = np.asarray(dst)
    cfg_mid, cfg_fin, plan, nc_mid, nc_fin = _get_built(src, dst)

    layers = [
        (np.asarray(W0), np.asarray(al0), np.asarray(ar0), np.asarray(b0)),
        (np.asarray(W1), np.asarray(al1), np.asarray(ar1), np.asarray(b1)),
        (np.asarray(W2), np.asarray(al2), np.asarray(ar2), np.asarray(b2)),
    ]
    h = np.zeros((cfg_mid.NPAD, _D), dtype=np.float32)
    h[:_N] = features
    for li, (W, al, ar, b) in enumerate(layers):
        final = li == 2
        cfg = cfg_fin if final else cfg_mid
        nc = nc_fin if final else nc_mid
        maps = _make_in_maps(cfg, plan, h, W, al, ar, b)
        res = run_bass_kernel_spmd(
            nc, maps, list(range(8)),
            trace=_collect_exec_ns is not None)
        if _collect_exec_ns is not None:
            _collect_exec_ns.append(res.exec_time_ns)
        outd = _DH if final else _D
        h = _assemble(cfg, res.results, outd)
    return h[:_N].astype(np.float32)


# revision 10
# speedup vs baseline: 1.0455x; 1.0455x over previous
"""GAT (3-layer, DGL GATConv-style) on 8 Trainium2 NeuronCores.

Self-contained kernel: kernel(**inputs) takes the full unsharded inputs
(features [50000,256] f32, src/dst [800000] i32, per-layer W/al/ar/b),
distributes across 8 cores (dst-slab graph partition), runs one Bass
kernel launch per GAT layer, and returns the full [50000, 64] output.

Device-side design (per core, per layer):
  phase A: each core computes the node table for its OWN slab only:
           row = [feat int8 x256 (per-row scale) | el f16 x4 |
                  scale f16 | er f16 x4] (274B in a 512B-stride row),
           in natural node order; feat is quantized to int8 on the
           scalar engine with a per-row reciprocal scale from the DVE
           absmax.  One 8-core DRAM AllGather then replicates the
           full 50176-row table to every core.
  phase B: per-edge rows gathered with dma_gather (266B payload =
           int8 feat + f16 el + f16 scale, int16 node-id indices into
           lo/hi table halves, 4 SWDGE queues);
           t = el[src]+er[dst]  (er via one-hot PE matmul, el via DVE);
           ex = max(exp(t), exp(0.2t))  == exp(leaky_relu(t, 0.2));
           weighted scatter-aggregation as PE matmul:
              psum[64dst, 260] += onehot_ed(fp8).T @ [feat*(ex*s) | ex]
           with the int8 dequant scale s folded into the edge weights;
           epilogue: rst = acc/den + (h+b); final layer computes
           mean_h relu(rst) via scalar-engine relu with scale=1/4.
Graph structure (tile schedule, one-hot matrices, gather indices) is
precomputed on the host once and reused for all three layers.
"""

import sys

sys.path.insert(0, "/opt/trn_rl_repo")

import inspect
import textwrap

import numpy as np
import ml_dtypes

import concourse.bacc as bacc
import concourse.bass as bass
import concourse.mybir as mybir
import concourse.tile as tile

F32 = mybir.dt.float32
F16 = mybir.dt.float16
BF16 = mybir.dt.bfloat16
F8 = mybir.dt.float8e4
I16 = mybir.dt.int16
I8 = mybir.dt.int8
U8 = mybir.dt.uint8

BF = ml_dtypes.bfloat16
E4M3 = ml_dtypes.float8_e4m3

# --- patch dma_gather: drop the (transpose-only) elem_size%256 assert ---
_src = textwrap.dedent(inspect.getsource(bass.BassGpSimd.dma_gather))
_src = _src.replace("elem_size_bytes > 0 and elem_size_bytes % 256 == 0",
                    "elem_size_bytes > 0")
_src = _src.replace("def dma_gather(", "def _dma_gather_relaxed(", 1)
_ns = dict(bass.__dict__)
exec(compile(_src, "patched_dma_gather", "exec"), _ns)
bass.BassGpSimd.dma_gather_relaxed = _ns["_dma_gather_relaxed"]


class Cfg:
    def __init__(self, N, E, D, H, DH, n_cores, win=64, kblk=16, grp=6,
                 out_heads_mean=False):
        self.N = N
        self.E = E
        self.D = D
        self.H = H
        self.DH = DH
        self.C = n_cores
        self.WIN = win      # dst nodes per window (psum group)
        self.KBLK = kblk    # edge-tiles per compute block
        self.GRP = grp      # windows per gather group
        slab = -(-N // n_cores)
        slab = -(-slab // win) * win
        while (slab * n_cores) % 128:
            slab += win
        self.NSLAB = slab
        self.NPAD = slab * n_cores
        self.NW = slab // win
        assert self.NPAD % 128 == 0
        assert self.NSLAB % 128 == 0
        self.NT = self.NPAD // 128
        self.TS = self.NSLAB // 128          # own-slab tiles per core
        self.HALF = min(32768, self.NPAD)    # lo table rows (node id < HALF)
        self.HIBASE = self.NPAD - self.HALF  # hi rows indexed from HIBASE
        self.WXC = D + 2 * H                 # Wx columns: feat | el | er
        self.ROWB = D + 2 * H + 2            # gathered payload bytes
        self.RSTB = 512                      # table row stride bytes
        self.out_heads_mean = out_heads_mean


def plan_edges(cfg, src, dst):
    """Common tile schedule + per-core edge tensors.

    Tiles are grouped: per window, lo(A)-half tiles then hi(B)-half
    tiles (half = which slab-half the src node's table row lives in);
    windows are grouped into gather groups of GRP windows.  Edges
    within a (core, window, half) segment are sorted by src row for
    gather locality.
    """
    C, WIN, NW, NSLAB, GRP = cfg.C, cfg.WIN, cfg.NW, cfg.NSLAB, cfg.GRP
    core_of = dst // NSLAB
    dloc = dst % NSLAB
    win_of = dloc // WIN

    deg = np.zeros(cfg.NPAD, dtype=np.int64)
    np.add.at(deg, dst, 1)
    zdeg = deg == 0

    half_of = (src >= cfg.HALF).astype(np.int64)  # 0 = lo, 1 = hi
    row_of = np.where(half_of == 0, src, src - cfg.HIBASE)

    # counts per (core, window, half)
    cnt = np.zeros((C, NW, 2), dtype=np.int64)
    np.add.at(cnt, (core_of, win_of, half_of), 1)
    # fake edges (src row 0 -> A) for zero-degree dsts
    zz = np.nonzero(zdeg)[0]
    np.add.at(cnt, (zz // NSLAB, (zz % NSLAB) // WIN, np.zeros(len(zz), np.int64)), 1)

    t_lo = -(-cnt[:, :, 0].max(axis=0) // 128)
    t_hi = -(-cnt[:, :, 1].max(axis=0) // 128)
    # every window needs >= 1 tile total (fakes guarantee lo>=1 when needed)
    t_lo = np.maximum(t_lo, (t_lo + t_hi == 0).astype(np.int64))

    # global slot ids: grouped by (group, half, window)
    wslots = [[] for _ in range(NW)]
    hslots = {}          # (w, half) -> list of slot ids
    groups = []
    T = 0
    for g in range(-(-NW // GRP)):
        ws = list(range(g * GRP, min((g + 1) * GRP, NW)))
        slots = []
        lo0 = T
        for w in ws:
            hslots[(w, 0)] = list(range(T, T + int(t_lo[w])))
            wslots[w] += hslots[(w, 0)]
            slots += [(w, 0)] * int(t_lo[w])
            T += int(t_lo[w])
        lo1 = T
        for w in ws:
            hslots[(w, 1)] = list(range(T, T + int(t_hi[w])))
            wslots[w] += hslots[(w, 1)]
            slots += [(w, 1)] * int(t_hi[w])
            T += int(t_hi[w])
        hi1 = T
        groups.append(dict(slots=slots, lo=(lo0, lo1), hi=(lo1, hi1)))

    eidx = np.zeros((C, 128, T * 8), dtype=np.int16)
    ohe = np.zeros((C, 128, T * WIN), dtype=E4M3)
    ohd = np.zeros((C, 64, T * 128), dtype=E4M3)

    key = (core_of * NW + win_of) * 2 + half_of
    order = np.lexsort((row_of, key))        # sort by src row within segments
    r_sorted = row_of[order]
    d_sorted = dst[order]
    cw = key[order]
    starts = np.searchsorted(cw, np.arange(C * NW * 2))
    ends = np.searchsorted(cw, np.arange(C * NW * 2) + 1)

    # wrap map: index i of a tile -> (row i%16, col i//16)
    wrap_r = np.arange(128) % 16
    wrap_c = np.arange(128) // 16

    for c in range(C):
        for w in range(NW):
            base_d = c * NSLAB + w * WIN
            for half in (0, 1):
                kk = (c * NW + w) * 2 + half
                i0, i1 = starts[kk], ends[kk]
                rr_ = list(r_sorted[i0:i1])
                dd = list((d_sorted[i0:i1] - base_d))
                if half == 0:
                    for dl in range(WIN):
                        if zdeg[base_d + dl]:
                            rr_.append(0)
                            dd.append(dl)
                sl_ids = hslots[(w, half)]
                nslots = len(sl_ids) * 128
                assert len(rr_) <= nslots, (c, w, half, len(rr_), nslots)
                npad = nslots - len(rr_)
                rr_ += [0] * npad
                dd += [-1] * npad
                rows = np.asarray(rr_, dtype=np.int64)
                dd = np.asarray(dd, dtype=np.int64)
                for j, t in enumerate(sl_ids):
                    rr = rows[j * 128:(j + 1) * 128]
                    ddj = dd[j * 128:(j + 1) * 128]
                    eidx[c, wrap_r, t * 8 + wrap_c] = rr.astype(np.int16)
                    p = np.nonzero(ddj >= 0)[0]
                    ohe[c, p, t * WIN + ddj[p]] = E4M3(1.0)
                    ohd[c, ddj[p], t * 128 + p] = E4M3(1.0)
    # replicate idx rows 0:16 across all 8 Q7 core groups
    for c in range(C):
        eidx[c] = np.tile(eidx[c, :16], (8, 1))
    return dict(groups=groups, wslots=wslots, T=T, eidx=eidx, ohe=ohe, ohd=ohd)


def pack_hT(cfg, h):
    """[NPAD, D] f32 -> [128, NT*D] f16 with cols (tile, kchunk, row):
    out[p, i*D + k*128 + q] = h[i*128 + q, k*128 + p]."""
    NT, D = cfg.NT, cfg.D
    KC = D // 128
    h4 = h.reshape(NT, 128, KC, 128).astype(np.float16)
    return np.ascontiguousarray(h4.transpose(3, 0, 2, 1).reshape(128, NT * D))


def make_wx(cfg, W, al, ar):
    H, DH = cfg.H, cfg.DH
    alm = np.zeros((cfg.D, H), dtype=np.float64)
    arm = np.zeros((cfg.D, H), dtype=np.float64)
    for h in range(H):
        alm[h * DH:(h + 1) * DH, h] = al[h]
        arm[h * DH:(h + 1) * DH, h] = ar[h]
    Wx = np.concatenate(
        [W.astype(np.float64), W.astype(np.float64) @ alm,
         W.astype(np.float64) @ arm], axis=1)
    return Wx.astype(np.float16)


def build_kernel(cfg, plan, final):
    D, H = cfg.D, cfg.H
    WIN, KBLK = cfg.WIN, cfg.KBLK
    ROWB, RSTB, WXC = cfg.ROWB, cfg.RSTB, cfg.WXC
    T = plan["T"]
    KC = D // 128
    DEN = D + H                            # 260 scatter columns
    # table row byte offsets: int8 feat | el f16 x4 | scale f16 | er f16 x4
    ELOFF = D                              # 256
    SCOFF = D + 2 * H                      # 264
    EROFF = D + 2 * H + 2                  # 266
    ERABS = 9                              # f16 slots el..er per row
    OUTD = cfg.DH if (cfg.out_heads_mean and final) else D

    nc = bacc.Bacc("TRN2", target_bir_lowering=False, debug=False,
                   enable_asserts=False, num_devices=cfg.C, num_swdge_queues=4)

    hTs = nc.dram_tensor("hTs", [128, cfg.TS * D], F16, kind="ExternalInput")
    Wx = nc.dram_tensor("Wx", [D, WXC], F16, kind="ExternalInput")
    hb = nc.dram_tensor("hb", [cfg.NSLAB, D], F16, kind="ExternalInput")
    eidx = nc.dram_tensor("eidx", [128, T * 8], I16, kind="ExternalInput")
    ohe_d = nc.dram_tensor("ohe", [128, T * WIN], F8, kind="ExternalInput")
    ohd_d = nc.dram_tensor("ohd", [64, T * 128], F8, kind="ExternalInput")
    out = nc.dram_tensor("out", [cfg.NSLAB, OUTD], F16, kind="ExternalOutput")
    tabS = nc.dram_tensor("tabS", [cfg.NSLAB, RSTB], U8, kind="Internal")
    tab = nc.dram_tensor("tab", [cfg.NPAD, RSTB], U8, kind="Internal",
                         addr_space="Shared")

    with tile.TileContext(nc) as tc:
        with (
            tc.tile_pool(name="const", bufs=1) as cpool,
            tc.tile_pool(name="grow", bufs=4) as gpool,
            tc.tile_pool(name="oh", bufs=6) as opool,
            tc.tile_pool(name="exg", bufs=3) as xpool,
            tc.tile_pool(name="tt", bufs=4) as tpool,
            tc.tile_pool(name="epi", bufs=4) as epool,
        ):
            wx0 = cpool.tile([128, WXC], F16, tag="wx0")
            wx1 = cpool.tile([128, WXC], F16, tag="wx1")
            nc.sync.dma_start(out=wx0[:], in_=Wx[0:128, :])
            nc.sync.dma_start(out=wx1[:], in_=Wx[128:256, :])
            eidx_t = cpool.tile([128, T * 8], I16, tag="eidx")
            nc.sync.dma_start(out=eidx_t[:], in_=eidx[:, :])

            # --- phase A: own slab, A-half then B-half ---
            erwin = cpool.tile([64, cfg.NW * H], F16, tag="erwin")
            with (
                tc.tile_pool(name="hblk", bufs=3) as hpool,
                tc.tile_pool(name="fblk", bufs=3) as fpool,
                tc.tile_pool(name="psA", bufs=2, space="PSUM") as psA,
            ):
                def phase_a(tabS, t0, nt):
                    AB = 8
                    tabS_r = tabS[:, :].rearrange("(i p) c -> p i c", p=128)
                    for blk0 in range(0, nt, AB):
                        blk1 = min(blk0 + AB, nt)
                        nb = blk1 - blk0
                        hblk = hpool.tile([128, AB * D], F16)
                        i0 = (t0 + blk0) * D
                        nc.sync.dma_start(
                            out=hblk[:, 0:nb * D], in_=hTs[:, i0:i0 + nb * D])
                        qblk = fpool.tile([128, AB * D], I8, tag="qblk")
                        eblk = fpool.tile([128, AB * ERABS], F16, tag="eblk")
                        for j in range(nb):
                            ps = psA.tile([128, WXC], F32)
                            for k in range(KC):
                                nc.tensor.matmul(
                                    out=ps[:],
                                    lhsT=hblk[:, j * D + k * 128:j * D + (k + 1) * 128],
                                    rhs=(wx0 if k == 0 else wx1)[:],
                                    start=(k == 0), stop=(k == KC - 1))
                            mx = fpool.tile([128, 1], F32, tag="mx")
                            nc.vector.tensor_reduce(
                                out=mx[:], in_=ps[:, 0:D],
                                axis=mybir.AxisListType.XYZW,
                                op=mybir.AluOpType.max,
                                apply_absolute_value=True)
                            ms = fpool.tile([128, 1], F32, tag="ms")
                            nc.vector.tensor_scalar_mul(
                                out=ms[:], in0=mx[:], scalar1=1.0 / 127.0)
                            rs = fpool.tile([128, 1], F32, tag="rs")
                            nc.vector.reciprocal(out=rs[:], in_=ms[:])
                            nc.scalar.activation(
                                out=qblk[:, j * D:(j + 1) * D],
                                in_=ps[:, 0:D],
                                func=mybir.ActivationFunctionType.Copy,
                                scale=rs[:])
                            nc.vector.tensor_copy(
                                out=eblk[:, j * ERABS + 4:j * ERABS + 5],
                                in_=ms[:])
                            nc.vector.tensor_copy(
                                out=eblk[:, j * ERABS:j * ERABS + 4],
                                in_=ps[:, D:D + H])
                            nc.vector.tensor_copy(
                                out=eblk[:, j * ERABS + 5:(j + 1) * ERABS],
                                in_=ps[:, D + H:D + 2 * H])
                        nc.sync.dma_start(
                            out=tabS_r[:, blk0:blk1, 0:D].bitcast(I8),
                            in_=qblk[:, 0:nb * D].rearrange("p (j c) -> p j c", c=D))
                        nc.sync.dma_start(
                            out=tabS_r[:, blk0:blk1, ELOFF:ELOFF + 2 * ERABS]
                            .bitcast(F16),
                            in_=eblk[:, 0:nb * ERABS].rearrange(
                                "p (j c) -> p j c", c=ERABS))

                phase_a(tabS, 0, cfg.TS)
                er_ap = (tabS[:, :].rearrange("(w d) c -> d w c", d=WIN)
                         [:, :, EROFF:EROFF + 2 * H].bitcast(F16))
                nc.sync.dma_start(
                    out=erwin[:].rearrange("p (w h) -> p w h", h=H),
                    in_=er_ap)
                nc.gpsimd.collective_compute(
                    kind="AllGather", op=mybir.AluOpType.bypass,
                    replica_groups=[list(range(cfg.C))],
                    ins=[tabS[:, :]], outs=[tab[:, :]])

            # --- phase B ---
            import contextlib
            _stk = contextlib.ExitStack()
            psT = _stk.enter_context(tc.tile_pool(name="psT", bufs=2, space="PSUM"))
            psB = _stk.enter_context(
                tc.tile_pool(name="psB", bufs=cfg.GRP, space="PSUM"))
            qn = [0]
            slot_to_win = {}
            for w in range(cfg.NW):
                for s in plan["wslots"][w]:
                    slot_to_win[s] = w

            hb_r = hb[:, :].rearrange("(w d) c -> d w c", d=WIN)
            out_r = out[:, :].rearrange("(w d) c -> d w c", d=WIN)
            for g, grp in enumerate(plan["groups"]):
                s_begin = grp["lo"][0]
                s_end = grp["hi"][1]
                nslot = s_end - s_begin
                w_lo = g * cfg.GRP
                w_hi = min((g + 1) * cfg.GRP, cfg.NW)
                nwg = w_hi - w_lo
                hbg = epool.tile([WIN, cfg.GRP * D], F16, tag="hbg")
                nc.sync.dma_start(
                    out=hbg[:, 0:nwg * D].rearrange("d (w c) -> d w c", c=D),
                    in_=hb_r[:, w_lo:w_hi, :])
                og = epool.tile([WIN, cfg.GRP * OUTD], F16, tag="og")
                grow = gpool.tile([128, nslot * ROWB], U8, tag="grow")
                CHUNK = 15  # tiles per gather call; small calls stay at
                            # pure desc-gen rate (no ring-reclaim stall)
                for half, (hh0, hh1) in (("lo", grp["lo"]), ("hi", grp["hi"])):
                    src_ap = (tab[0:cfg.HALF, 0:ROWB] if half == "lo"
                              else tab[cfg.HIBASE:cfg.NPAD, 0:ROWB])
                    for h0 in range(hh0, hh1, CHUNK):
                        h1 = min(h0 + CHUNK, hh1)
                        ni = (h1 - h0) * 128
                        nc.gpsimd.dma_gather_relaxed(
                            out_ap=grow[:, (h0 - s_begin) * ROWB:(h1 - s_begin) * ROWB]
                            .rearrange("p (t e) -> p t e", e=ROWB),
                            in_ap=src_ap,
                            idxs_ap=eidx_t[:, h0 * 8:h1 * 8],
                            num_idxs=ni, num_idxs_reg=ni,
                            elem_size=ROWB, elem_step=RSTB,
                            single_packet=False, queue_num=qn[0] % 4)
                        qn[0] += 1

                accs = {}
                open_w = {}
                for b0 in range(s_begin, s_end, KBLK):
                    b1 = min(b0 + KBLK, s_end)
                    k = b1 - b0
                    ohe_b = opool.tile([128, KBLK * WIN], F8, tag="ohe")
                    nc.scalar.dma_start(
                        out=ohe_b[:, 0:k * WIN],
                        in_=ohe_d[:, b0 * WIN:b1 * WIN])
                    ohd_b = opool.tile([64, KBLK * 128], F8, tag="ohd")
                    nc.scalar.dma_start(
                        out=ohd_b[:, 0:k * 128],
                        in_=ohd_d[:, b0 * 128:b1 * 128])
                    pst = psT.tile([128, KBLK * H], F32)
                    for j in range(k):
                        s = b0 + j
                        w = slot_to_win[s]
                        nc.tensor.matmul(
                            out=pst[:, j * H:(j + 1) * H],
                            lhsT=ohd_b[:, j * 128:(j + 1) * 128],
                            rhs=erwin[:, w * H:(w + 1) * H],
                            start=True, stop=True, skip_group_check=True)
                    grow_k = (grow[:, (b0 - s_begin) * ROWB:(b1 - s_begin) * ROWB]
                              .rearrange("p (t e) -> p t e", e=ROWB))
                    tsrc = tpool.tile([128, KBLK * H], BF16, tag="tt")
                    nc.vector.tensor_add(
                        out=tsrc[:, 0:k * H].rearrange("p (k h) -> p k h", h=H),
                        in0=pst[:, 0:k * H].rearrange("p (k h) -> p k h", h=H),
                        in1=grow_k[:, :, ELOFF:ELOFF + 2 * H].bitcast(F16))
                    xa = tpool.tile([128, KBLK * H], BF16, tag="xa")
                    xb = tpool.tile([128, KBLK * H], BF16, tag="xb")
                    nc.scalar.activation(
                        out=xa[:, 0:k * H], in_=tsrc[:, 0:k * H],
                        func=mybir.ActivationFunctionType.Exp)
                    nc.scalar.activation(
                        out=xb[:, 0:k * H], in_=tsrc[:, 0:k * H],
                        func=mybir.ActivationFunctionType.Exp, scale=0.2)
                    exg = xpool.tile([128, KBLK * DEN], BF16, tag="exg")
                    exg_k = exg[:, 0:k * DEN].rearrange("p (k c) -> p k c", c=DEN)
                    nc.vector.tensor_max(
                        out=exg_k[:, :, D:DEN],
                        in0=xa[:, 0:k * H].rearrange("p (k h) -> p k h", h=H),
                        in1=xb[:, 0:k * H].rearrange("p (k h) -> p k h", h=H))
                    # exs = ex * per-row scale  (dequant folded into weights)
                    exs = tpool.tile([128, KBLK * H], BF16, tag="exs")
                    nc.vector.tensor_mul(
                        out=exs[:, 0:k * H].rearrange("p (k h) -> p k h", h=H),
                        in0=exg_k[:, :, D:DEN],
                        in1=grow_k[:, :, SCOFF:SCOFF + 2].bitcast(F16)
                        .to_broadcast([128, k, H]))
                    feat_in = grow_k[:, :, 0:D].bitcast(I8).rearrange(
                        "p k (h f) -> p k h f", f=cfg.DH)
                    ex_in = (exs[:, 0:k * H].rearrange("p (k h) -> p k h", h=H)
                             .to_broadcast([128, k, H, cfg.DH]))
                    exg_out = exg_k[:, :, 0:D].rearrange(
                        "p k (h f) -> p k h f", f=cfg.DH)
                    nc.vector.tensor_mul(out=exg_out, in0=feat_in, in1=ex_in)

                    # scatter matmuls for this block
                    for j in range(k):
                        s = b0 + j
                        w = slot_to_win[s]
                        if w not in accs:
                            acc_w = psB.tile([WIN, DEN], F32, tag="acc")
                            accs[w] = acc_w
                            open_w[w] = 0
                        first = open_w[w] == 0
                        last = s == plan["wslots"][w][-1]
                        open_w[w] += 1
                        nc.tensor.matmul(
                            out=accs[w][:],
                            lhsT=ohe_b[:, j * WIN:(j + 1) * WIN],
                            rhs=exg[:, j * DEN:(j + 1) * DEN],
                            start=first, stop=last, skip_group_check=True)
                        if last:
                            acc = accs.pop(w)
                            wl = w - w_lo
                            rec = epool.tile([WIN, H], F32, tag="rec")
                            nc.vector.reciprocal(out=rec[:], in_=acc[:, D:DEN])
                            rst = epool.tile([WIN, D], F32, tag="rst")
                            for hh in range(H):
                                nc.scalar.activation(
                                    out=rst[:, hh * cfg.DH:(hh + 1) * cfg.DH],
                                    in_=acc[:, hh * cfg.DH:(hh + 1) * cfg.DH],
                                    func=mybir.ActivationFunctionType.Copy,
                                    scale=rec[:, hh:hh + 1])
                            if cfg.out_heads_mean and final:
                                nc.vector.tensor_add(
                                    out=rst[:], in0=rst[:],
                                    in1=hbg[:, wl * D:(wl + 1) * D])
                                rq = epool.tile([WIN, D], BF16, tag="rq")
                                nc.scalar.activation(
                                    out=rq[:], in_=rst[:],
                                    func=mybir.ActivationFunctionType.Relu,
                                    scale=0.25)
                                o1 = epool.tile([WIN, cfg.DH], F32, tag="o1")
                                nc.vector.tensor_add(
                                    out=o1[:], in0=rq[:, 0:cfg.DH],
                                    in1=rq[:, cfg.DH:2 * cfg.DH])
                                o2 = epool.tile([WIN, cfg.DH], F32, tag="o2")
                                nc.vector.tensor_add(
                                    out=o2[:], in0=rq[:, 2 * cfg.DH:3 * cfg.DH],
                                    in1=rq[:, 3 * cfg.DH:4 * cfg.DH])
                                nc.vector.tensor_add(
                                    out=og[:, wl * OUTD:(wl + 1) * OUTD],
                                    in0=o1[:], in1=o2[:])
                            else:
                                nc.vector.tensor_add(
                                    out=og[:, wl * OUTD:(wl + 1) * OUTD],
                                    in0=rst[:],
                                    in1=hbg[:, wl * D:(wl + 1) * D])
                nc.sync.dma_start(
                    out=out_r[:, w_lo:w_hi, :],
                    in_=og[:, 0:nwg * OUTD].rearrange("d (w c) -> d w c", c=OUTD))
            _stk.close()

    nc.compile()
    return nc


# ---------------------------------------------------------------------------
# kernel() entry point
# ---------------------------------------------------------------------------
_CACHE = {}

_N, _E, _D, _H, _DH = 50000, 800000, 256, 4, 64


def _get_built(src, dst):
    key = "built"
    if key in _CACHE:
        return _CACHE[key]
    cfg_mid = Cfg(_N, _E, _D, _H, _DH, n_cores=8, out_heads_mean=False)
    cfg_fin = Cfg(_N, _E, _D, _H, _DH, n_cores=8, out_heads_mean=True)
    plan = plan_edges(cfg_mid, src.astype(np.int64), dst.astype(np.int64))
    nc_mid = build_kernel(cfg_mid, plan, final=False)
    nc_fin = build_kernel(cfg_fin, plan, final=True)
    _CACHE[key] = (cfg_mid, cfg_fin, plan, nc_mid, nc_fin)
    return _CACHE[key]


def _make_in_maps(cfg, plan, h, W, al, ar, b):
    hTp = pack_hT(cfg, h)
    Wx = make_wx(cfg, W, al, ar)
    maps = []
    for c in range(cfg.C):
        sl = slice(c * cfg.NSLAB, (c + 1) * cfg.NSLAB)
        hb = (h[sl] + b[None, :]).astype(np.float16)
        hTs = hTp[:, c * cfg.TS * cfg.D:(c + 1) * cfg.TS * cfg.D]
        maps.append(dict(hTs=hTs, Wx=Wx, hb=hb, eidx=plan["eidx"][c],
                         ohe=plan["ohe"][c], ohd=plan["ohd"][c]))
    return maps


def _assemble(cfg, results, outd):
    out = np.zeros((cfg.NPAD, outd), dtype=np.float32)
    for c in range(cfg.C):
        out[c * cfg.NSLAB:(c + 1) * cfg.NSLAB] = results[c]["out"].astype(np.float32)
    out[cfg.N:] = 0.0
    return out


def kernel(features, src, dst, W0, al0, ar0, b0, W1, al1, ar1, b1,
           W2, al2, ar2, b2, _collect_exec_ns=None):
    from concourse.bass_utils import run_bass_kernel_spmd

    features = np.asarray(features, dtype=np.float32)
    src = np.asarray(src)
    dst = np.asarray(dst)
    cfg_mid, cfg_fin, plan, nc_mid, nc_fin = _get_built(src, dst)

    layers = [
        (np.asarray(W0), np.asarray(al0), np.asarray(ar0), np.asarray(b0)),
        (np.asarray(W1), np.asarray(al1), np.asarray(ar1), np.asarray(b1)),
        (np.asarray(W2), np.asarray(al2), np.asarray(ar2), np.asarray(b2)),
    ]
    h = np.zeros((cfg_mid.NPAD, _D), dtype=np.float32)
    h[:_N] = features
    for li, (W, al, ar, b) in enumerate(layers):
        final = li == 2
        cfg = cfg_fin if final else cfg_mid
        nc = nc_fin if final else nc_mid
        maps = _make_in_maps(cfg, plan, h, W, al, ar, b)
        res = run_bass_kernel_spmd(
            nc, maps, list(range(8)),
            trace=_collect_exec_ns is not None)
        if _collect_exec_ns is not None:
            _collect_exec_ns.append(res.exec_time_ns)
        outd = _DH if final else _D
        h = _assemble(cfg, res.results, outd)
    return h[:_N].astype(np.float32)
